# revision 1
# baseline (speedup 1.0000x reference)
"""MoE feed-forward (top-2 of 8 experts) Trainium2 Bass kernel.

Strategy: data-parallel over tokens (8 cores x 2048 tokens). Each core:
  1. Router on device in fp32 (logits -> top-2 -> renormalized gates).
  2. Builds, per expert, a compacted token-index list fully on device
     (prefix-sums via PE matmuls with a triangular matrix + equality-mask
     matmuls to emit the index lists).
  3. Gathers the selected token rows with indirect DMA (bf16), PE-transposes
     them to feature-major, runs gate/up/down matmuls in bf16 (fp32 PSUM
     accumulation), scales by the fp32 gate, and scatter-adds the result
     rows back to the output with accumulate-DMA.

Host side only stages/shards inputs (dtype cast + row permutation) and
re-assembles the output shards.
"""

import sys

sys.path.insert(0, "/opt/trn_rl_repo")

import numpy as np
import ml_dtypes

import concourse.bass as bass
import concourse.bacc as bacc
import concourse.tile as tile
import concourse.mybir as mybir
from concourse.bass import ts, ds

F32 = mybir.dt.float32
BF16 = mybir.dt.bfloat16
I32 = mybir.dt.int32
I16 = mybir.dt.int16
USE_CUSTOM_DMA = True
AX = mybir.AxisListType
OP = mybir.AluOpType
ACT = mybir.ActivationFunctionType

# Problem shapes (hardcoded per contest contract)
N_CORES = 8
B, T, H, I, E = 4, 4096, 1024, 2048, 8
BT = B * T            # 16384 tokens
TPC = BT // N_CORES   # 2048 tokens per core
CAP = 640             # per-(core, expert) slot capacity (mean 512, sigma ~21)
TAIL_IF = False

KH = H // 128         # 8  h-dim chunks
KI = I // 128         # 16 i-dim chunks


def _nb_splits(cap):
    """Split cap into <=512 column blocks for gate/up matmuls."""
    out, o = [], 0
    while o < cap:
        n = min(512, cap - o)
        out.append((o, n))
        o += n
    return out


def build_program(tpc=TPC, cap=CAP, debug=False, custom=USE_CUSTOM_DMA):
    """Build the per-core SPMD Bass program (identical on all 8 cores)."""
    C = tpc // 128       # token chunks (16)
    NCH = cap // 128     # capacity chunks per expert (5)
    NBS = _nb_splits(cap)

    nc = bacc.Bacc("TRN2", target_bir_lowering=False, debug=debug)

    # ---- per-core external inputs -------------------------------------
    hT = nc.dram_tensor("hT", [H, tpc], F32, kind="ExternalInput")
    hrow = nc.dram_tensor("hrow", [tpc + 1, H], BF16, kind="ExternalInput")
    gwT = nc.dram_tensor("gwT", [H, E], F32, kind="ExternalInput")
    wg_d = nc.dram_tensor("wgt", [E, I // 128, 128, KH, 128], BF16,
                          kind="ExternalInput")
    wu_d = nc.dram_tensor("wut", [E, I // 128, 128, KH, 128], BF16,
                          kind="ExternalInput")
    wd_d = nc.dram_tensor("wd", [E, I, H], BF16, kind="ExternalInput")
    # constants
    tri_d = nc.dram_tensor("tri", [128, 128], F32, kind="ExternalInput")
    rid_d = nc.dram_tensor("rid", [128, C], F32, kind="ExternalInput")
    io128_d = nc.dram_tensor("io128", [128, 128], F32, kind="ExternalInput")
    ioN_d = nc.dram_tensor("ioN", [128, NCH], F32, kind="ExternalInput")
    id128_d = nc.dram_tensor("id128", [128, 128], BF16, kind="ExternalInput")
    ones1_d = nc.dram_tensor("ones1", [128, 1], F32, kind="ExternalInput")
    capW = cap // 16
    if custom:
        io16r_d = nc.dram_tensor("io16r", [128, 128], F32, kind="ExternalInput")
        ioW_d = nc.dram_tensor("ioW", [128, capW], F32, kind="ExternalInput")

    out_d = nc.dram_tensor("out", [tpc + 1, H], F32, kind="ExternalOutput")
    cwtab = nc.dram_tensor("cwtab", [tpc + 1, 64], F32)  # internal scratch

    wg_r = wg_d[:].rearrange("e t ki ko i -> e t ki ko i")
    wu_r = wu_d[:].rearrange("e t ki ko i -> e t ki ko i")
    wd_r = wd_d[:].rearrange("e (ko ki) h -> e ki ko h", ki=128)
    hT_r = hT[:].rearrange("(ko ki) t -> ki ko t", ki=128)
    gwT_r = gwT[:].rearrange("(ko ki) e -> ki ko e", ki=128)

    with tile.TileContext(nc) as tc:
        with (
            tc.tile_pool(name="const", bufs=1) as pconst,
            tc.tile_pool(name="persist", bufs=1) as ppers,
        ):
            # constants into SBUF
            tri_sb = pconst.tile([128, 128], F32)
            nc.sync.dma_start(tri_sb[:], tri_d[:])
            rid_sb = pconst.tile([128, C], F32)
            nc.sync.dma_start(rid_sb[:], rid_d[:])
            io128_sb = pconst.tile([128, 128], F32)
            nc.sync.dma_start(io128_sb[:], io128_d[:])
            ioN_sb = pconst.tile([128, NCH], F32)
            nc.sync.dma_start(ioN_sb[:], ioN_d[:])
            id128_sb = pconst.tile([128, 128], BF16)
            nc.sync.dma_start(id128_sb[:], id128_d[:])
            if custom:
                io16r_sb = pconst.tile([128, 128], F32)
                nc.sync.dma_start(io16r_sb[:], io16r_d[:])
                ioW_sb = pconst.tile([128, capW], F32)
                nc.sync.dma_start(ioW_sb[:], ioW_d[:])
            ones1_sb = pconst.tile([128, 1], F32)
            nc.sync.dma_start(ones1_sb[:], ones1_d[:])
            zw_sb = pconst.tile([128, 128], BF16)
            nc.vector.memset(zw_sb[:], 0.0)

            zt = pconst.tile([128, H], F32)
            nc.vector.memset(zt[:], 0.0)

            # persistent routing products
            cw_tm = ppers.tile([128, C, E], F32)      # gates (0 if not picked)
            if custom:
                idx16 = ppers.tile([128, E, capW], I16)  # wrapped row-ids
            else:
                idx32 = ppers.tile([128, E, NCH], I32)   # token row-ids
            cnts_i = ppers.tile([1, E], I32)             # per-expert counts

            # ============ phase 1+2: router + routing math ==============
            with (
                tc.tile_pool(name="rt", bufs=2) as prt,
                tc.tile_pool(name="rt1", bufs=1) as prt1,
                tc.tile_pool(name="rtps", bufs=2, space="PSUM") as prtps,
            ):
                gwT_sb = prt1.tile([128, KH, E], F32)
                nc.sync.dma_start(gwT_sb[:], gwT_r)
                hT_sb = prt1.tile([128, KH, tpc], F32)
                ntb = max(1, tpc // 512)
                for tb in range(ntb):
                    nc.scalar.dma_start(
                        hT_sb[:, :, ts(tb, tpc // ntb)],
                        hT_r[:, :, ts(tb, tpc // ntb)],
                    )

                L_tm = prt1.tile([128, C, E], F32)
                for c in range(C):
                    ps_l = prtps.tile([128, E], F32, tag="psl")
                    for k in range(KH):
                        nc.tensor.matmul(
                            ps_l[:],
                            lhsT=hT_sb[:, k, ts(c, 128)],
                            rhs=gwT_sb[:, k, :],
                            start=(k == 0),
                            stop=(k == KH - 1),
                        )
                    nc.vector.tensor_copy(L_tm[:, c, :], ps_l[:])

                # top-2 + renormalized gates, all token-major [128, C, E]
                m1 = prt1.tile([128, C], F32)
                nc.vector.reduce_max(m1[:], L_tm[:], axis=AX.X)
                m1b = m1[:, :, None].to_broadcast([128, C, E])
                ismax = prt.tile([128, C, E], F32)
                nc.vector.tensor_tensor(ismax[:], L_tm[:], m1b, op=OP.is_ge)
                tmp = prt.tile([128, C, E], F32)
                nc.vector.tensor_scalar_mul(tmp[:], ismax[:], 1e30)
                lm = prt.tile([128, C, E], F32)
                nc.vector.tensor_tensor(lm[:], L_tm[:], tmp[:], op=OP.subtract)
                m2 = prt1.tile([128, C], F32)
                nc.vector.reduce_max(m2[:], lm[:], axis=AX.X)
                sel = ppers.tile([128, C, E], F32)
                nc.vector.tensor_tensor(
                    sel[:], L_tm[:], m2[:, :, None].to_broadcast([128, C, E]),
                    op=OP.is_ge,
                )
                lshift = prt.tile([128, C, E], F32)
                nc.vector.tensor_tensor(lshift[:], L_tm[:], m1b, op=OP.subtract)
                ex = prt.tile([128, C, E], F32)
                nc.scalar.activation(ex[:], lshift[:], ACT.Exp)
                gun = prt.tile([128, C, E], F32)
                nc.vector.tensor_tensor(gun[:], ex[:], sel[:], op=OP.mult)
                den = prt1.tile([128, C], F32)
                nc.vector.reduce_sum(den[:], gun[:], axis=AX.X)
                rec = prt1.tile([128, C], F32)
                nc.vector.reciprocal(rec[:], den[:])
                nc.vector.tensor_tensor(
                    cw_tm[:], gun[:], rec[:, :, None].to_broadcast([128, C, E]),
                    op=OP.mult,
                )

                # zero-init output (scatter-add target) + gate table; all
                # on the scalar HWDGE ring, after the hT chunks
                for c in range(C):
                    nc.scalar.dma_start(out_d[ts(c, 128), :], zt[:])
                nc.scalar.dma_start(out_d[tpc : tpc + 1, :], zt[:1, :])
                nc.scalar.dma_start(cwtab[tpc : tpc + 1, :], zt[:1, :64])
                with nc.allow_non_contiguous_dma(reason="small strided cw table"):
                    nc.scalar.dma_start(
                        cwtab[0:tpc, 0:E].rearrange("(q c) e -> q c e", q=128),
                        cw_tm[:],
                    )

            # ===== phases 3+4: index build + MLP, software-pipelined =====
            with (
                tc.tile_pool(name="ix", bufs=2) as pix,
                tc.tile_pool(name="xg", bufs=2) as pxg,
                tc.tile_pool(name="xr", bufs=2) as pxr,
                tc.tile_pool(name="wgt", bufs=6) as pwgt,
                tc.tile_pool(name="wdp", bufs=2) as pwdp,
                tc.tile_pool(name="act", bufs=2) as pact,
                tc.tile_pool(name="ev", bufs=3) as pev,
                tc.tile_pool(name="yp", bufs=2) as pyp,
                tc.tile_pool(name="gups", bufs=2, space="PSUM") as pgu,
                tc.tile_pool(name="yps", bufs=2, space="PSUM") as pyps,
                tc.tile_pool(name="trps", bufs=2, space="PSUM") as ptr,
            ):

                def emit_idx(e):
                    sel_e = pix.tile([128, C], F32, tag="sel_e")
                    nc.vector.tensor_copy(sel_e[:], sel[:, :, e])
                    rsel = pix.tile([128, C], F32, tag="rsel")
                    nc.vector.tensor_tensor(rsel[:], rid_sb[:], sel_e[:], op=OP.mult)

                    rowtot = pix.tile([128, 1], F32, tag="rowtot")
                    nc.vector.reduce_sum(rowtot[:], sel_e[:], axis=AX.X)
                    ps_off = ptr.tile([128, 1], F32, tag="pst")
                    nc.tensor.matmul(
                        ps_off[:], lhsT=tri_sb[:], rhs=rowtot[:],
                        start=True, stop=True,
                    )
                    rowoff = pix.tile([128, 1], F32, tag="rowoff")
                    nc.vector.tensor_copy(rowoff[:], ps_off[:])
                    ps_cnt = ptr.tile([1, 1], F32, tag="pst")
                    nc.tensor.matmul(
                        ps_cnt[:], lhsT=ones1_sb[:], rhs=rowtot[:],
                        start=True, stop=True,
                    )
                    nc.vector.tensor_copy(cnts_i[0:1, e : e + 1], ps_cnt[:])

                    # exclusive prefix along the C free positions
                    a = pix.tile([128, C], F32, tag="pfxa")
                    nc.vector.memset(a[:], 0.0)
                    if C > 1:
                        nc.vector.tensor_copy(a[:, 1:C], sel_e[:, 0 : C - 1])
                    s = 1
                    while s < C:
                        b = pix.tile([128, C], F32, tag=f"pfxb{s}")
                        nc.vector.tensor_copy(b[:, 0:s], a[:, 0:s])
                        nc.vector.tensor_tensor(
                            b[:, s:C], a[:, s:C], a[:, 0 : C - s], op=OP.add
                        )
                        a = b
                        s *= 2
                    slot = pix.tile([128, C], F32, tag="slot")
                    nc.vector.tensor_scalar_add(slot[:], a[:], rowoff[:, 0:1])

                    slot_i = pix.tile([128, C], I32, tag="sloti")
                    nc.vector.tensor_copy(slot_i[:], slot[:])

                    WW = 16 if custom else 128
                    NW = capW if custom else NCH
                    iomask_sb = io16r_sb if custom else io128_sb
                    iocol_sb = ioW_sb if custom else ioN_sb
                    smod_i = pix.tile([128, C], I32, tag="smodi")
                    nc.vector.tensor_scalar(
                        smod_i[:], slot_i[:], WW - 1, None, op0=OP.bitwise_and
                    )
                    sdiv_i = pix.tile([128, C], I32, tag="sdivi")
                    nc.vector.tensor_scalar(
                        sdiv_i[:], slot_i[:], WW.bit_length() - 1, None,
                        op0=OP.logical_shift_right,
                    )
                    smod = pix.tile([128, C], F32, tag="smod")
                    nc.vector.tensor_copy(smod[:], smod_i[:])
                    sdiv = pix.tile([128, C], F32, tag="sdiv")
                    nc.vector.tensor_copy(sdiv[:], sdiv_i[:])

                    ps_idx = ptr.tile([128, NW], F32, tag="pst")
                    ps_cov = ptr.tile([128, NW], F32, tag="pst")
                    for c in range(C):
                        mask = pix.tile([128, 128], F32, tag="mask")
                        nc.vector.tensor_tensor(
                            mask[:],
                            smod[:, c : c + 1].to_broadcast([128, 128]),
                            iomask_sb[:],
                            op=OP.is_equal,
                        )
                        lhs1 = pix.tile([128, 128], F32, tag="lhs1")
                        nc.vector.tensor_tensor(
                            lhs1[:], mask[:],
                            rsel[:, c : c + 1].to_broadcast([128, 128]),
                            op=OP.mult,
                        )
                        lhs2 = pix.tile([128, 128], F32, tag="lhs2")
                        nc.vector.tensor_tensor(
                            lhs2[:], mask[:],
                            sel_e[:, c : c + 1].to_broadcast([128, 128]),
                            op=OP.mult,
                        )
                        rhsm = pix.tile([128, NW], F32, tag="rhsm")
                        nc.vector.tensor_tensor(
                            rhsm[:],
                            sdiv[:, c : c + 1].to_broadcast([128, NW]),
                            iocol_sb[:],
                            op=OP.is_equal,
                        )
                        nc.tensor.matmul(
                            ps_idx[:], lhsT=lhs1[:], rhs=rhsm[:],
                            start=(c == 0), stop=(c == C - 1),
                        )
                        nc.tensor.matmul(
                            ps_cov[:], lhsT=lhs2[:], rhs=rhsm[:],
                            start=(c == 0), stop=(c == C - 1),
                        )
                    # empty slots -> dump row (tpc)
                    t1 = pix.tile([128, NW], F32, tag="t1")
                    nc.vector.tensor_scalar(
                        t1[:], ps_cov[:], -float(tpc), float(tpc),
                        op0=OP.mult, op1=OP.add,
                    )
                    idxf = pix.tile([128, NW], F32, tag="idxf")
                    nc.vector.tensor_tensor(idxf[:], ps_idx[:], t1[:], op=OP.add)
                    if custom:
                        nc.vector.tensor_copy(idx16[:, e, :], idxf[:])
                    else:
                        nc.vector.tensor_copy(idx32[:, e, :], idxf[:])

                def emit_gather(e):
                    cwg = pxg.tile([128, NCH, 64], F32, tag="cwg")
                    if custom:
                        xT = pxg.tile([128, KH, cap], BF16, tag="xT")
                        nc.gpsimd.dma_gather(
                            out_ap=xT[:], in_ap=hrow[:],
                            idxs_ap=idx16[:, e, :],
                            num_idxs=cap, num_idxs_reg=cap,
                            elem_size=H, transpose=True,
                        )
                        nc.gpsimd.dma_gather(
                            out_ap=cwg[:], in_ap=cwtab[:],
                            idxs_ap=idx16[:, e, :],
                            num_idxs=cap, num_idxs_reg=cap,
                            elem_size=64, transpose=False,
                        )
                        return xT, cwg
                    xrows = pxr.tile([128, NCH, H], BF16, tag="xrows")
                    for k in range(NCH):
                        nc.gpsimd.indirect_dma_start(
                            out=xrows[:, k, :],
                            out_offset=None,
                            in_=hrow[:],
                            in_offset=bass.IndirectOffsetOnAxis(
                                ap=idx32[:, e, k : k + 1], axis=0
                            ),
                        )
                        nc.gpsimd.indirect_dma_start(
                            out=cwg[:, k, :],
                            out_offset=None,
                            in_=cwtab[:],
                            in_offset=bass.IndirectOffsetOnAxis(
                                ap=idx32[:, e, k : k + 1], axis=0
                            ),
                        )
                    return xrows, cwg

                emit_idx(0)
                pending = emit_gather(0)
                # down-proj weights prefetch one expert ahead on the scalar
                # HWDGE ring (idle during gate/up), double-buffered
                wd_next = pwdp.tile([128, KI, H], BF16, tag="wd_sb")
                nc.scalar.dma_start(wd_next[:], wd_r[0])
                for e in range(E):
                    xrl, cwg = pending
                    wd_sb = wd_next
                    rv = None
                    if TAIL_IF and cap > 512:
                        creg = nc.alloc_register(mybir.EngineType.PE, f"cnt{e}")
                        nc.tensor.reg_load(creg, cnts_i[0:1, e : e + 1])
                        rv = bass.RuntimeValue(creg)
                    if custom:
                        xT = xrl
                    else:
                        # ---- transpose gathered rows to feature-major ----
                        xT = pxg.tile([128, KH, cap], BF16, tag="xT")
                        for k in range(NCH):
                            for ko in range(KH):
                                ps_t = ptr.tile([128, 128], BF16, tag="pst")
                                nc.tensor.transpose(
                                    ps_t[:], xrl[:, k, ts(ko, 128)], id128_sb[:]
                                )
                                nc.vector.tensor_copy(
                                    xT[:, ko, ts(k, 128)], ps_t[:]
                                )
                    cwcol = pxg.tile([128, NCH], F32, tag="cwcol")
                    nc.vector.tensor_copy(cwcol[:], cwg[:, :, e])

                    # ---- gate/up proj + silu*up ----
                    actT = pact.tile([128, KI, cap], BF16, tag="actT")
                    for ic in range(KI):
                        wg_t = pwgt.tile([128, KH, 128], BF16, tag="wg_t")
                        nc.sync.dma_start(wg_t[:], wg_r[e, ic])
                        wu_t = pwgt.tile([128, KH, 128], BF16, tag="wu_t")
                        nc.sync.dma_start(wu_t[:], wu_r[e, ic])
                        for n0, nsz in NBS:
                            ps_g = pgu.tile([128, 512], F32, tag="psg")
                            ps_u = pgu.tile([128, 512], F32, tag="psu")

                            def _gu_mms(n0=n0, nsz=nsz, ps_g=ps_g, ps_u=ps_u,
                                        wg_t=wg_t, wu_t=wu_t, xT=xT):
                                for k in range(KH):
                                    nc.tensor.matmul(
                                        ps_g[:, :nsz],
                                        lhsT=wg_t[:, k, :],
                                        rhs=xT[:, k, ds(n0, nsz)],
                                        start=(k == 0), stop=(k == KH - 1),
                                    )
                                for k in range(KH):
                                    nc.tensor.matmul(
                                        ps_u[:, :nsz],
                                        lhsT=wu_t[:, k, :],
                                        rhs=xT[:, k, ds(n0, nsz)],
                                        start=(k == 0), stop=(k == KH - 1),
                                    )

                            if rv is not None and n0 >= 512:
                                with tc.If(
                                    rv > n0, preferred_fallthrough_block=True
                                ) as cmp:
                                    _gu_mms()
                                with cmp.Else():
                                    nc.tensor.matmul(
                                        ps_g[:, :nsz], lhsT=zw_sb[:],
                                        rhs=xT[:, 0, ds(n0, nsz)],
                                        start=True, stop=True,
                                    )
                                    nc.tensor.matmul(
                                        ps_u[:, :nsz], lhsT=zw_sb[:],
                                        rhs=xT[:, 0, ds(n0, nsz)],
                                        start=True, stop=True,
                                    )
                            else:
                                _gu_mms()
                            s_sb = pev.tile([128, 512], F32, tag="s_sb")
                            nc.scalar.activation(
                                s_sb[:, :nsz], ps_g[:, :nsz], ACT.Silu
                            )
                            nc.vector.tensor_tensor(
                                actT[:, ic, ds(n0, nsz)],
                                s_sb[:, :nsz], ps_u[:, :nsz], op=OP.mult,
                            )

                    # next expert's index list: emitted here so its PE
                    # micro-matmuls land ~60us after their DVE mask inputs
                    # are produced (no PE stall), and its gathers go on the
                    # gpsimd ring before this expert's scatters
                    if e + 1 < E:
                        emit_idx(e + 1)
                        pending = emit_gather(e + 1)

                    # ---- down proj (token-major out) + gate scale ----
                    if e + 1 < E:
                        wd_next = pwdp.tile([128, KI, H], BF16, tag="wd_sb")
                        nc.scalar.dma_start(wd_next[:], wd_r[e + 1])
                    for m in range(NCH):
                        y_sb = pyp.tile([128, H], F32, tag="y_sb")
                        for hb in range(2):
                            ps_y = pyps.tile([128, 512], F32, tag="psy")

                            def _dn_mms(m=m, hb=hb, ps_y=ps_y, actT=actT,
                                        wd_sb=wd_sb):
                                for k in range(KI):
                                    nc.tensor.matmul(
                                        ps_y[:],
                                        lhsT=actT[:, k, ts(m, 128)],
                                        rhs=wd_sb[:, k, ts(hb, 512)],
                                        start=(k == 0), stop=(k == KI - 1),
                                    )

                            if rv is not None and m * 128 >= 512:
                                with tc.If(
                                    rv > m * 128,
                                    preferred_fallthrough_block=True,
                                ) as cmp:
                                    _dn_mms()
                                with cmp.Else():
                                    nc.tensor.matmul(
                                        ps_y[:], lhsT=zw_sb[:],
                                        rhs=wd_sb[:, 0, ts(hb, 512)],
                                        start=True, stop=True,
                                    )
                            else:
                                _dn_mms()
                            nc.scalar.mul(
                                y_sb[:, ts(hb, 512)], ps_y[:],
                                mul=cwcol[:, m : m + 1],
                            )
                        # scatter row-chunk m as soon as it is scaled
                        if custom:
                            nc.gpsimd.dma_scatter_add(
                                out_d[:],
                                y_sb[:, None, :],
                                idx16[:, e, ts(m, 8)],
                                128, 128, H,
                            )
                        else:
                            nc.gpsimd.indirect_dma_start(
                                out=out_d[:],
                                out_offset=bass.IndirectOffsetOnAxis(
                                    ap=idx32[:, e, m : m + 1], axis=0
                                ),
                                in_=y_sb[:],
                                in_offset=None,
                                compute_op=OP.add,
                            )

    nc.compile()
    return nc


# ======================= host staging =================================

def _consts(tpc, cap):
    C = tpc // 128
    NCH = cap // 128
    tri = (np.arange(128)[:, None] < np.arange(128)[None, :]).astype(np.float32)
    rid = (np.arange(128)[:, None] * C + np.arange(C)[None, :]).astype(np.float32)
    io128 = np.broadcast_to(np.arange(128, dtype=np.float32), (128, 128)).copy()
    ioN = np.broadcast_to(np.arange(NCH, dtype=np.float32), (128, NCH)).copy()
    id128 = np.eye(128, dtype=np.float32).astype(ml_dtypes.bfloat16)
    ones1 = np.ones((128, 1), dtype=np.float32)
    io16r = np.broadcast_to(np.arange(128) % 16, (128, 128)).astype(np.float32)
    capW = cap // 16
    ioW = np.broadcast_to(np.arange(capW, dtype=np.float32), (128, capW)).copy()
    return tri, rid, io128, ioN, id128, io16r, ioW, ones1


def make_in_maps(hidden_states, gate_w, wg, wu, wd, tpc=TPC, cap=CAP,
                 n_cores=N_CORES):
    h = np.asarray(hidden_states, dtype=np.float32).reshape(-1, H)
    gate_w = np.asarray(gate_w, dtype=np.float32)
    bf = ml_dtypes.bfloat16

    def _retile_up(w):  # [E,H,I] -> [E, I/128, ki=128, KH, 128]
        w = np.asarray(w, dtype=np.float32).astype(bf)
        w = w.reshape(E, KH, 128, I // 128, 128)      # e, ko, ki, t, icol
        return np.ascontiguousarray(w.transpose(0, 3, 2, 1, 4))

    wg_b = _retile_up(wg)
    wu_b = _retile_up(wu)
    wd_b = np.asarray(wd, dtype=np.float32).astype(bf)
    gwT = np.ascontiguousarray(gate_w.T)
    tri, rid, io128, ioN, id128, io16r, ioW, ones1 = _consts(tpc, cap)

    C = tpc // 128
    in_maps = []
    for c in range(n_cores):
        shard = h[c * tpc : (c + 1) * tpc]             # [tpc, H] token j order
        hT = np.ascontiguousarray(shard.T)             # [H, tpc]
        # row r = q*C + c  <->  token j = c*128 + q
        hperm = np.ascontiguousarray(
            shard.reshape(C, 128, H).swapaxes(0, 1).reshape(tpc, H)
        )
        hrow = np.zeros((tpc + 1, H), dtype=bf)
        hrow[:tpc] = hperm.astype(bf)
        in_maps.append({
            "hT": hT, "hrow": hrow, "gwT": gwT,
            "wgt": wg_b, "wut": wu_b, "wd": wd_b,
            "tri": tri, "rid": rid, "io128": io128, "ioN": ioN,
            "id128": id128, "io16r": io16r, "ioW": ioW,
            "ones1": ones1,
        })
    return in_maps


def assemble_output(results, tpc=TPC, n_cores=N_CORES):
    C = tpc // 128
    shards = []
    for c in range(n_cores):
        o = np.asarray(results[c]["out"])[:tpc]        # drop dump row
        # invert permutation: token j = c*128+q lives at row q*C+c
        shards.append(o.reshape(128, C, H).swapaxes(0, 1).reshape(tpc, H))
    return np.concatenate(shards, axis=0).reshape(B, T, H)


_PROGRAM_CACHE = {}


def run(hidden_states, gate_w, wg, wu, wd, trace=False, trace_kwargs=None):
    from concourse.bass_utils import run_bass_kernel_spmd

    key = (TPC, CAP)
    if key not in _PROGRAM_CACHE:
        _PROGRAM_CACHE[key] = build_program(TPC, CAP)
    nc = _PROGRAM_CACHE[key]
    in_maps = make_in_maps(hidden_states, gate_w, wg, wu, wd)
    res = run_bass_kernel_spmd(
        nc, in_maps, core_ids=list(range(N_CORES)),
        trace=trace, **(trace_kwargs or {}),
    )
    return assemble_output(res.results), res


def kernel(hidden_states, gate_w, wg, wu, wd):
    out, _ = run(hidden_states, gate_w, wg, wu, wd)
    return out

